# revision 26
# baseline (speedup 1.0000x reference)
"""Trainium2 Bass kernel for nn_DistillLoss (ragged KL distillation loss).

Strategy (data-parallel over batch, 8 NeuronCores):
  - Host: shard B=1024 samples into 8 x 128. Per core, samples are permuted
    so per-SDMA-engine gather bytes are balanced (partition granule of 4 ->
    engine 2*((p%32)//4) + p//64), with the known-slow engine 15 derated.
  - Device (per core): ragged doc segments are fetched straight from the
    core's contiguous doc-row slice with "super-row" indirect DMAs: one
    index per sample gathers `blk` consecutive doc rows per descriptor and
    casts f32 -> bf16 inline (SWDGE). Hybrid schedule: blk=8 for the dense
    first 64 doc slots, blk=4 for the ragged tail (less round-up waste).
  - Dot products: per gather call, a wide DVE tensor_tensor multiply (bf16
    2x mode) feeds `a` Scalar-engine activation-Copy free-dim accumulates;
    the remaining slots run fused on DVE via scalar_tensor_tensor (1x).
    All accumulating ops are hardware-capped at 1x, so the work is split
    ~60/40 between ACT and DVE to keep both under the DMA roofline.
  - Masked log-softmax + KL epilogue on [128, 128] f32 tiles; per-sample
    losses DMA out; host sums across samples and cores.
"""

import sys

sys.path.insert(0, "/opt/trn_rl_repo")

import numpy as np

NCORES = 8
B = 1024
D = 1024
M = 128
BL = B // NCORES  # 128 samples per core
TEMP = 0.02
NEG = -1e30
OOB = np.int32(2**30)

_CACHE = {}


def _schedule(dense_blk=8, dense_m=64, tail_blk=4):
    """Gather schedule: list of (m0, blk, act_slots) per indirect call."""
    sched = []
    for m0 in range(0, dense_m, dense_blk):
        sched.append((m0, dense_blk, 5))
    i = 0
    for m0 in range(dense_m, M, tail_blk):
        sched.append((m0, tail_blk, 3 if i % 2 == 0 else 2))
        i += 1
    return tuple(sched)


def _build_nc(sched=(), rrows=1, bufs=3, use_bf16=True):
    from concourse import bacc, bass, mybir, tile

    f32 = mybir.dt.float32
    bf16 = mybir.dt.bfloat16
    u8 = mybir.dt.uint8
    i32 = mybir.dt.int32
    ALU = mybir.AluOpType
    AF = mybir.ActivationFunctionType
    AX = mybir.AxisListType
    ddt = bf16 if use_bf16 else f32

    ncalls = len(sched)
    max_a = max(a for _, _, a in sched)

    nc = bacc.Bacc("TRN2", target_bir_lowering=False, debug=False, num_devices=NCORES)

    rdocs = nc.dram_tensor("rdocs", [rrows, D], f32, kind="ExternalInput").ap()
    idxs = nc.dram_tensor("idxs", [BL, ncalls], i32, kind="ExternalInput").ap()
    q = nc.dram_tensor("q", [BL, D], f32, kind="ExternalInput").ap()
    traw = nc.dram_tensor("traw", [BL, M], f32, kind="ExternalInput").ap()
    mask = nc.dram_tensor("mask", [BL, M], u8, kind="ExternalInput").ap()
    out = nc.dram_tensor("out", [BL, 1], f32, kind="ExternalOutput").ap()

    from contextlib import ExitStack

    with tile.TileContext(nc) as tc, ExitStack() as ctx:
        consts = ctx.enter_context(tc.tile_pool(name="consts", bufs=1))
        dbig = ctx.enter_context(tc.tile_pool(name="dbig", bufs=bufs))
        dsmall = ctx.enter_context(tc.tile_pool(name="dsmall", bufs=bufs + 1))
        scratch = ctx.enter_context(tc.tile_pool(name="scratch", bufs=4))
        sc2_act = ctx.enter_context(tc.tile_pool(name="actout", bufs=4))
        small = ctx.enter_context(tc.tile_pool(name="small", bufs=1))

        # first gather's index column goes in a tiny DMA so descriptors can
        # be emitted as early as possible
        idx0_sb = consts.tile([BL, 1], i32)
        nc.sync.dma_start(out=idx0_sb, in_=idxs[:, 0:1])
        idx_sb = consts.tile([BL, ncalls], i32)
        nc.sync.dma_start(out=idx_sb, in_=idxs)
        traw_sb = consts.tile([BL, M], f32)
        nc.scalar.dma_start(out=traw_sb, in_=traw)
        mask_sb = consts.tile([BL, M], u8)
        nc.scalar.dma_start(out=mask_sb, in_=mask)
        negt = consts.tile([BL, M], f32)
        nc.vector.memset(negt, NEG)

        # q loads f32 via HWDGE; cast to bf16 on DVE so the Q7/SWDGE path
        # has nothing to do before the first gather's descriptors.
        qf_sb = consts.tile([BL, D], f32)
        nc.scalar.dma_start(out=qf_sb, in_=q)
        q_sb = consts.tile([BL, D], ddt)
        nc.vector.tensor_copy(q_sb, qf_sb)

        # q replicated for the widest per-call multiply
        qrep = consts.tile([BL, max_a * D], ddt)
        for r in range(max_a):
            nc.vector.tensor_copy(qrep[:, r * D : (r + 1) * D], q_sb)

        sim_bm = consts.tile([BL, M], f32)  # raw (unscaled) dots

        for ci, (m0, blk, a) in enumerate(sched):
            # NOTE: out AP must be 2D - a 3D out tile miscompiles the
            # indirect descriptors (probe3). Full 128-partition gathers keep
            # descriptors balanced; OOB-skipped descriptors are ~free.
            pool_ = dbig if blk >= 8 else dsmall
            dtile = pool_.tile([BL, blk * D], ddt,
                               tag="dbig" if blk >= 8 else "dsmall")
            nc.gpsimd.indirect_dma_start(
                out=dtile,
                out_offset=None,
                in_=rdocs,
                in_offset=bass.IndirectOffsetOnAxis(
                    ap=(idx0_sb[:, 0:1] if ci == 0 else idx_sb[:, ci : ci + 1]),
                    axis=0,
                ),
                bounds_check=rrows - 1,
                oob_is_err=False,
            )
            # one wide bf16 multiply (DVE 2x) feeds `a` ACT accumulates;
            # remaining slots run fused on DVE (1x).
            sc = scratch.tile([BL, max_a * D], ddt, tag="sc")
            if a > 0:
                nc.vector.tensor_mul(
                    sc[:, : a * D], dtile[:, : a * D], qrep[:, : a * D]
                )
            for j in range(a):
                m = m0 + j
                aout = sc2_act.tile([BL, D], ddt, tag="aout")
                # AF.Identity is a lossy spline (probe4); AF.Copy is exact.
                nc.scalar.activation(
                    aout, sc[:, j * D : (j + 1) * D], AF.Copy,
                    accum_out=sim_bm[:, m : m + 1],
                )
            for j in range(a, blk):
                m = m0 + j
                scf = scratch.tile([BL, D], ddt, tag="scf")
                nc.vector.scalar_tensor_tensor(
                    out=scf,
                    in0=dtile[:, j * D : (j + 1) * D],
                    scalar=1.0,
                    in1=q_sb,
                    op0=ALU.mult,
                    op1=ALU.mult,
                    accum_out=sim_bm[:, m : m + 1],
                )

        # ---- epilogue on [b=128, m=128] f32 tiles ----
        simt = small.tile([BL, M], f32)
        nc.vector.tensor_scalar(simt, sim_bm, 1.0 / TEMP, None, op0=ALU.mult)
        simm = small.tile([BL, M], f32)
        nc.vector.select(simm, mask_sb, simt, negt)

        nmx = small.tile([BL, 1], f32)
        nc.vector.tensor_reduce(nmx, simm, axis=AX.X, op=ALU.max, negate=True)
        shifted = small.tile([BL, M], f32)
        nc.vector.tensor_scalar_add(shifted, simm, nmx[:, 0:1])

        e_sb = small.tile([BL, M], f32)
        s_sb = small.tile([BL, 1], f32)
        nc.scalar.activation(e_sb, shifted, AF.Exp, accum_out=s_sb)
        logs = small.tile([BL, 1], f32)
        nc.scalar.activation(logs, s_sb, AF.Ln)

        tsum = small.tile([BL, 1], f32)
        nc.vector.tensor_reduce(tsum, traw_sb, axis=AX.X, op=ALU.add)
        denom = small.tile([BL, 1], f32)
        nc.vector.tensor_scalar_add(denom, tsum, 1e-9)
        rec = small.tile([BL, 1], f32)
        nc.vector.reciprocal(rec, denom)
        tn = small.tile([BL, M], f32)
        nc.vector.tensor_scalar_mul(tn, traw_sb, rec[:, 0:1])
        sumtn = small.tile([BL, 1], f32)
        nc.vector.tensor_mul(sumtn, tsum, rec)

        iszero = small.tile([BL, M], f32)
        nc.vector.tensor_scalar(iszero, tn, 0.0, None, op0=ALU.is_le)
        tsafe = small.tile([BL, M], f32)
        nc.vector.tensor_add(tsafe, tn, iszero)
        logt = small.tile([BL, M], f32)
        nc.scalar.activation(logt, tsafe, AF.Ln)

        sc2 = small.tile([BL, M], f32)
        term1 = small.tile([BL, 1], f32)
        nc.vector.scalar_tensor_tensor(
            out=sc2, in0=tn, scalar=1.0, in1=logt,
            op0=ALU.mult, op1=ALU.mult, accum_out=term1,
        )
        sc3 = small.tile([BL, M], f32)
        term2 = small.tile([BL, 1], f32)
        nc.vector.scalar_tensor_tensor(
            out=sc3, in0=tn, scalar=1.0, in1=shifted,
            op0=ALU.mult, op1=ALU.mult, accum_out=term2,
        )

        lgs = small.tile([BL, 1], f32)
        nc.vector.tensor_mul(lgs, logs, sumtn)
        kc = small.tile([BL, 1], f32)
        nc.vector.tensor_sub(kc, term1, term2)
        nc.vector.tensor_add(kc, kc, lgs)
        nc.sync.dma_start(out=out, in_=kc)

    nc.compile()
    return nc


def _get_nc(**cfg):
    key = ("nc",) + tuple(sorted(cfg.items()))
    if key not in _CACHE:
        _CACHE[key] = _build_nc(**cfg)
    return _CACHE[key]


def _make_in_maps(query_embeds, doc_embeds, soft_labels, num_docs_per_sample,
                  bufs=3, use_bf16=True, dense_blk=8, dense_m=64, tail_blk=4):
    qf = np.ascontiguousarray(np.asarray(query_embeds, dtype=np.float32))
    de = np.ascontiguousarray(np.asarray(doc_embeds, dtype=np.float32))
    sl = np.ascontiguousarray(np.asarray(soft_labels, dtype=np.float32))
    nd = np.asarray(num_docs_per_sample).astype(np.int64)
    total = de.shape[0]

    sched = _schedule(dense_blk, dense_m, tail_blk)
    max_blk = max(blk for _, blk, _ in sched)

    offs = np.zeros(B, np.int64)
    offs[1:] = np.cumsum(nd)[:-1]
    # effective (clipped) doc counts, mirroring the reference's clip behaviour
    nde = np.minimum(np.minimum(nd, M), np.maximum(total - offs, 0))
    mask = (np.arange(M)[None, :] < nde[:, None]).astype(np.float32)
    traw = sl * mask

    # per-core contiguous doc-row slices
    base = np.empty(NCORES, np.int64)
    rows = np.empty(NCORES, np.int64)
    for c in range(NCORES):
        s0, s1 = c * BL, (c + 1) * BL - 1
        base[c] = offs[s0]
        rows[c] = offs[s1] + nde[s1] - base[c]
    rrows = int(rows.max()) + max_blk  # padding rows for block overreads

    # per-call block-start indices; OOB sentinel when the call's slot range
    # is entirely past the sample's doc count
    m0s = np.array([m0 for m0, _, _ in sched], np.int64)[None, :]  # [1, ncalls]
    relp = (offs - np.repeat(base, BL))[:, None] + m0s  # [B, ncalls]
    valid = m0s < nde[:, None]
    idx_all = np.where(valid, relp, OOB).astype(np.int32)

    # per-sample fetched rows (for engine load balancing)
    blks = np.array([blk for _, blk, _ in sched], np.int64)[None, :]
    w_all = np.where(valid, np.minimum(blks, nde[:, None] - m0s), 0).sum(axis=1)

    # Each gather descriptor for partition p lands on SDMA engine
    # 2*((p%32)//4) + p//64. Permute samples per core so per-engine bytes are
    # even, giving the known-slow engine 15 a lighter share.
    rate = np.ones(16)
    rate[15] = 0.72
    gran_eng = np.array([2 * (g % 8) + g // 16 for g in range(32)])
    eng_parts = {e: [] for e in range(16)}
    for g in range(32):
        eng_parts[gran_eng[g]].extend(range(4 * g, 4 * g + 4))

    perm = np.empty(B, np.int64)
    for c in range(NCORES):
        s0 = c * BL
        order = np.argsort(-w_all[s0 : s0 + BL], kind="stable")
        load = np.zeros(16)
        cap = np.full(16, 8)
        assign = {e: [] for e in range(16)}
        for i in order:
            scaled = (load + w_all[s0 + i]) / rate
            scaled[cap == 0] = np.inf
            e = int(np.argmin(scaled))
            assign[e].append(i)
            load[e] += w_all[s0 + i]
            cap[e] -= 1
        for e in range(16):
            for slot, i in enumerate(assign[e]):
                perm[s0 + eng_parts[e][slot]] = s0 + i

    in_maps = []
    for c in range(NCORES):
        s = slice(c * BL, (c + 1) * BL)
        p = perm[s]
        docs_c = np.zeros((rrows, D), np.float32)
        docs_c[: rows[c]] = de[base[c] : base[c] + rows[c]]
        in_maps.append(
            {
                "rdocs": docs_c,
                "idxs": np.ascontiguousarray(idx_all[p]),
                "q": np.ascontiguousarray(qf[p]),
                "traw": np.ascontiguousarray(traw[p]),
                "mask": np.ascontiguousarray(mask[p].astype(np.uint8)),
            }
        )
    cfg = {"sched": sched, "rrows": rrows, "bufs": bufs, "use_bf16": use_bf16}
    return in_maps, cfg


def run(in_maps, cfg=None, trace=False):
    from concourse import bass_utils

    nc = _get_nc(**(cfg or {}))
    return bass_utils.run_bass_kernel_spmd(
        nc, in_maps, list(range(NCORES)), trace=trace
    )


def kernel(query_embeds, doc_embeds, soft_labels, num_docs_per_sample):
    in_maps, cfg = _make_in_maps(
        query_embeds, doc_embeds, soft_labels, num_docs_per_sample
    )
    res = run(in_maps, cfg=cfg)
    tot = sum(float(r["out"].sum()) for r in res.results)
    return np.asarray(tot / B, dtype=np.float32)


# revision 30
# speedup vs baseline: 1.0528x; 1.0528x over previous
"""Trainium2 Bass kernel for nn_DistillLoss (ragged KL distillation loss).

Strategy (data-parallel over batch, 8 NeuronCores):
  - Host: shard B=1024 samples into 8 x 128. Per core, samples are permuted
    so per-SDMA-engine gather bytes are balanced (partition granule of 4 ->
    engine 2*((p%32)//4) + p//64), with the known-slow engine 15 derated.
  - Device (per core): ragged doc segments are fetched straight from the
    core's contiguous doc-row slice with "super-row" indirect DMAs: one
    index per sample gathers `blk` consecutive doc rows per descriptor and
    casts f32 -> bf16 inline (SWDGE). Hybrid schedule: blk=8 for the dense
    first 64 doc slots, blk=4 for the ragged tail (less round-up waste).
  - Dot products: per gather call, a wide DVE tensor_tensor multiply (bf16
    2x mode) feeds `a` Scalar-engine activation-Copy free-dim accumulates;
    the remaining slots run fused on DVE via scalar_tensor_tensor (1x).
    All accumulating ops are hardware-capped at 1x, so the work is split
    ~60/40 between ACT and DVE to keep both under the DMA roofline.
  - Masked log-softmax + KL epilogue on [128, 128] f32 tiles; per-sample
    losses DMA out; host sums across samples and cores.
"""

import sys

sys.path.insert(0, "/opt/trn_rl_repo")

import numpy as np

NCORES = 8
B = 1024
D = 1024
M = 128
BL = B // NCORES  # 128 samples per core
TEMP = 0.02
NEG = -1e30
OOB = np.int32(2**30)

_CACHE = {}


def _schedule(dense_blk=8, dense_m=64, tail_blk=4):
    """Gather schedule: list of (m0, blk, act_slots) per indirect call."""
    sched = []
    for m0 in range(0, dense_m, dense_blk):
        sched.append((m0, dense_blk, 5))
    i = 0
    for m0 in range(dense_m, M, tail_blk):
        sched.append((m0, tail_blk, 3 if i % 2 == 0 else 2))
        i += 1
    return tuple(sched)


def _build_nc(sched=(), rrows=1, bufs=3, use_bf16=True):
    from concourse import bacc, bass, mybir, tile

    f32 = mybir.dt.float32
    bf16 = mybir.dt.bfloat16
    u8 = mybir.dt.uint8
    i32 = mybir.dt.int32
    ALU = mybir.AluOpType
    AF = mybir.ActivationFunctionType
    AX = mybir.AxisListType
    ddt = bf16 if use_bf16 else f32

    ncalls = len(sched)
    max_a = max(a for _, _, a in sched)

    nc = bacc.Bacc("TRN2", target_bir_lowering=False, debug=False, num_devices=NCORES)

    rdocs = nc.dram_tensor("rdocs", [rrows, D], f32, kind="ExternalInput").ap()
    idxs = nc.dram_tensor("idxs", [BL, ncalls], i32, kind="ExternalInput").ap()
    q = nc.dram_tensor("q", [BL, D], f32, kind="ExternalInput").ap()
    traw = nc.dram_tensor("traw", [BL, M], f32, kind="ExternalInput").ap()
    mask = nc.dram_tensor("mask", [BL, M], u8, kind="ExternalInput").ap()
    out = nc.dram_tensor("out", [BL, 1], f32, kind="ExternalOutput").ap()

    from contextlib import ExitStack

    with tile.TileContext(nc) as tc, ExitStack() as ctx:
        consts = ctx.enter_context(tc.tile_pool(name="consts", bufs=1))
        dbig = ctx.enter_context(tc.tile_pool(name="dbig", bufs=bufs))
        dsmall = ctx.enter_context(tc.tile_pool(name="dsmall", bufs=2 * bufs - 2))
        scratch = ctx.enter_context(tc.tile_pool(name="scratch", bufs=3))
        sc2_act = ctx.enter_context(tc.tile_pool(name="actout", bufs=3))
        small = ctx.enter_context(tc.tile_pool(name="small", bufs=1))

        # first gather's index column goes in a tiny DMA so descriptors can
        # be emitted as early as possible
        idx0_sb = consts.tile([BL, 1], i32)
        nc.sync.dma_start(out=idx0_sb, in_=idxs[:, 0:1])
        idx_sb = consts.tile([BL, ncalls], i32)
        nc.sync.dma_start(out=idx_sb, in_=idxs)
        traw_sb = consts.tile([BL, M], f32)
        nc.scalar.dma_start(out=traw_sb, in_=traw)
        mask_sb = consts.tile([BL, M], u8)
        nc.scalar.dma_start(out=mask_sb, in_=mask)
        negt = consts.tile([BL, M], f32)
        nc.vector.memset(negt, NEG)

        # q loads f32 via HWDGE; cast to bf16 on DVE so the Q7/SWDGE path
        # has nothing to do before the first gather's descriptors.
        qf_sb = consts.tile([BL, D], f32)
        nc.scalar.dma_start(out=qf_sb, in_=q)
        q_sb = consts.tile([BL, D], ddt)
        nc.vector.tensor_copy(q_sb, qf_sb)

        # q replicated for the widest per-call multiply
        qrep = consts.tile([BL, max_a * D], ddt)
        for r in range(max_a):
            nc.vector.tensor_copy(qrep[:, r * D : (r + 1) * D], q_sb)

        sim_bm = consts.tile([BL, M], f32)  # raw (unscaled) dots

        for ci, (m0, blk, a) in enumerate(sched):
            # NOTE: out AP must be 2D - a 3D out tile miscompiles the
            # indirect descriptors (probe3). Full 128-partition gathers keep
            # descriptors balanced; OOB-skipped descriptors are ~free.
            pool_ = dbig if blk >= 8 else dsmall
            dtile = pool_.tile([BL, blk * D], ddt,
                               tag="dbig" if blk >= 8 else "dsmall")
            nc.gpsimd.indirect_dma_start(
                out=dtile,
                out_offset=None,
                in_=rdocs,
                in_offset=bass.IndirectOffsetOnAxis(
                    ap=(idx0_sb[:, 0:1] if ci == 0 else idx_sb[:, ci : ci + 1]),
                    axis=0,
                ),
                bounds_check=rrows - 1,
                oob_is_err=False,
            )
            # one wide bf16 multiply (DVE 2x) feeds `a` ACT accumulates;
            # remaining slots run fused on DVE (1x).
            sc = scratch.tile([BL, max_a * D], ddt, tag="sc")
            if a > 0:
                nc.vector.tensor_mul(
                    sc[:, : a * D], dtile[:, : a * D], qrep[:, : a * D]
                )
            for j in range(a):
                m = m0 + j
                aout = sc2_act.tile([BL, D], ddt, tag="aout")
                # AF.Identity is a lossy spline (probe4); AF.Copy is exact.
                nc.scalar.activation(
                    aout, sc[:, j * D : (j + 1) * D], AF.Copy,
                    accum_out=sim_bm[:, m : m + 1],
                )
            for j in range(a, blk):
                m = m0 + j
                scf = sc2_act.tile([BL, D], ddt, tag="scf")
                nc.vector.scalar_tensor_tensor(
                    out=scf,
                    in0=dtile[:, j * D : (j + 1) * D],
                    scalar=1.0,
                    in1=q_sb,
                    op0=ALU.mult,
                    op1=ALU.mult,
                    accum_out=sim_bm[:, m : m + 1],
                )

        # ---- epilogue on [b=128, m=128] f32 tiles ----
        simt = small.tile([BL, M], f32)
        nc.vector.tensor_scalar(simt, sim_bm, 1.0 / TEMP, None, op0=ALU.mult)
        simm = small.tile([BL, M], f32)
        nc.vector.select(simm, mask_sb, simt, negt)

        nmx = small.tile([BL, 1], f32)
        nc.vector.tensor_reduce(nmx, simm, axis=AX.X, op=ALU.max, negate=True)
        shifted = small.tile([BL, M], f32)
        nc.vector.tensor_scalar_add(shifted, simm, nmx[:, 0:1])

        e_sb = small.tile([BL, M], f32)
        s_sb = small.tile([BL, 1], f32)
        nc.scalar.activation(e_sb, shifted, AF.Exp, accum_out=s_sb)
        logs = small.tile([BL, 1], f32)
        nc.scalar.activation(logs, s_sb, AF.Ln)

        tsum = small.tile([BL, 1], f32)
        nc.vector.tensor_reduce(tsum, traw_sb, axis=AX.X, op=ALU.add)
        denom = small.tile([BL, 1], f32)
        nc.vector.tensor_scalar_add(denom, tsum, 1e-9)
        rec = small.tile([BL, 1], f32)
        nc.vector.reciprocal(rec, denom)
        tn = small.tile([BL, M], f32)
        nc.vector.tensor_scalar_mul(tn, traw_sb, rec[:, 0:1])
        sumtn = small.tile([BL, 1], f32)
        nc.vector.tensor_mul(sumtn, tsum, rec)

        iszero = small.tile([BL, M], f32)
        nc.vector.tensor_scalar(iszero, tn, 0.0, None, op0=ALU.is_le)
        tsafe = small.tile([BL, M], f32)
        nc.vector.tensor_add(tsafe, tn, iszero)
        logt = small.tile([BL, M], f32)
        nc.scalar.activation(logt, tsafe, AF.Ln)

        sc2 = small.tile([BL, M], f32)
        term1 = small.tile([BL, 1], f32)
        nc.vector.scalar_tensor_tensor(
            out=sc2, in0=tn, scalar=1.0, in1=logt,
            op0=ALU.mult, op1=ALU.mult, accum_out=term1,
        )
        sc3 = small.tile([BL, M], f32)
        term2 = small.tile([BL, 1], f32)
        nc.vector.scalar_tensor_tensor(
            out=sc3, in0=tn, scalar=1.0, in1=shifted,
            op0=ALU.mult, op1=ALU.mult, accum_out=term2,
        )

        lgs = small.tile([BL, 1], f32)
        nc.vector.tensor_mul(lgs, logs, sumtn)
        kc = small.tile([BL, 1], f32)
        nc.vector.tensor_sub(kc, term1, term2)
        nc.vector.tensor_add(kc, kc, lgs)
        nc.sync.dma_start(out=out, in_=kc)

    nc.compile()
    return nc


def _get_nc(**cfg):
    key = ("nc",) + tuple(sorted(cfg.items()))
    if key not in _CACHE:
        _CACHE[key] = _build_nc(**cfg)
    return _CACHE[key]


def _make_in_maps(query_embeds, doc_embeds, soft_labels, num_docs_per_sample,
                  bufs=4, use_bf16=True, dense_blk=8, dense_m=64, tail_blk=4):
    qf = np.ascontiguousarray(np.asarray(query_embeds, dtype=np.float32))
    de = np.ascontiguousarray(np.asarray(doc_embeds, dtype=np.float32))
    sl = np.ascontiguousarray(np.asarray(soft_labels, dtype=np.float32))
    nd = np.asarray(num_docs_per_sample).astype(np.int64)
    total = de.shape[0]

    sched = _schedule(dense_blk, dense_m, tail_blk)
    max_blk = max(blk for _, blk, _ in sched)

    offs = np.zeros(B, np.int64)
    offs[1:] = np.cumsum(nd)[:-1]
    # effective (clipped) doc counts, mirroring the reference's clip behaviour
    nde = np.minimum(np.minimum(nd, M), np.maximum(total - offs, 0))
    mask = (np.arange(M)[None, :] < nde[:, None]).astype(np.float32)
    traw = sl * mask

    # per-core contiguous doc-row slices
    base = np.empty(NCORES, np.int64)
    rows = np.empty(NCORES, np.int64)
    for c in range(NCORES):
        s0, s1 = c * BL, (c + 1) * BL - 1
        base[c] = offs[s0]
        rows[c] = offs[s1] + nde[s1] - base[c]
    rrows = int(rows.max()) + max_blk  # padding rows for block overreads

    # per-call block-start indices; OOB sentinel when the call's slot range
    # is entirely past the sample's doc count
    m0s = np.array([m0 for m0, _, _ in sched], np.int64)[None, :]  # [1, ncalls]
    relp = (offs - np.repeat(base, BL))[:, None] + m0s  # [B, ncalls]
    valid = m0s < nde[:, None]
    idx_all = np.where(valid, relp, OOB).astype(np.int32)

    # per-sample fetched rows (for engine load balancing); a valid call
    # always fetches the full blk rows (round-up overreads into padding)
    blks = np.array([blk for _, blk, _ in sched], np.int64)[None, :]
    w_all = (valid * blks).sum(axis=1)

    # Each gather descriptor for partition p lands on SDMA engine
    # 2*((p%32)//4) + p//64. Permute samples per core so per-engine bytes are
    # even, giving the known-slow engine 15 a lighter share.
    rate = np.ones(16)
    rate[15] = 0.72
    gran_eng = np.array([2 * (g % 8) + g // 16 for g in range(32)])
    eng_parts = {e: [] for e in range(16)}
    for g in range(32):
        eng_parts[gran_eng[g]].extend(range(4 * g, 4 * g + 4))

    perm = np.empty(B, np.int64)
    for c in range(NCORES):
        s0 = c * BL
        order = np.argsort(-w_all[s0 : s0 + BL], kind="stable")
        load = np.zeros(16)
        cap = np.full(16, 8)
        assign = {e: [] for e in range(16)}
        for i in order:
            scaled = (load + w_all[s0 + i]) / rate
            scaled[cap == 0] = np.inf
            e = int(np.argmin(scaled))
            assign[e].append(i)
            load[e] += w_all[s0 + i]
            cap[e] -= 1
        for e in range(16):
            for slot, i in enumerate(assign[e]):
                perm[s0 + eng_parts[e][slot]] = s0 + i

    in_maps = []
    for c in range(NCORES):
        s = slice(c * BL, (c + 1) * BL)
        p = perm[s]
        docs_c = np.zeros((rrows, D), np.float32)
        docs_c[: rows[c]] = de[base[c] : base[c] + rows[c]]
        in_maps.append(
            {
                "rdocs": docs_c,
                "idxs": np.ascontiguousarray(idx_all[p]),
                "q": np.ascontiguousarray(qf[p]),
                "traw": np.ascontiguousarray(traw[p]),
                "mask": np.ascontiguousarray(mask[p].astype(np.uint8)),
            }
        )
    cfg = {"sched": sched, "rrows": rrows, "bufs": bufs, "use_bf16": use_bf16}
    return in_maps, cfg


def run(in_maps, cfg=None, trace=False):
    from concourse import bass_utils

    nc = _get_nc(**(cfg or {}))
    return bass_utils.run_bass_kernel_spmd(
        nc, in_maps, list(range(NCORES)), trace=trace
    )


def kernel(query_embeds, doc_embeds, soft_labels, num_docs_per_sample):
    in_maps, cfg = _make_in_maps(
        query_embeds, doc_embeds, soft_labels, num_docs_per_sample
    )
    res = run(in_maps, cfg=cfg)
    tot = sum(float(r["out"].sum()) for r in res.results)
    return np.asarray(tot / B, dtype=np.float32)


# revision 35
# speedup vs baseline: 1.0607x; 1.0075x over previous
"""Trainium2 Bass kernel for nn_DistillLoss (ragged KL distillation loss).

Strategy (data-parallel over batch, 8 NeuronCores):
  - Host: shard B=1024 samples into 8 x 128. Per core, samples are permuted
    so per-SDMA-engine gather bytes are balanced (partition granule of 4 ->
    engine 2*((p%32)//4) + p//64), with the known-slow engine 15 derated.
  - Device (per core): ragged doc segments are fetched straight from the
    core's contiguous doc-row slice with "super-row" indirect DMAs: one
    index per sample gathers `blk` consecutive doc rows per descriptor and
    casts f32 -> bf16 inline (SWDGE). Hybrid schedule: blk=8 for the dense
    first 64 doc slots, blk=4 for the ragged tail (less round-up waste).
  - Dot products: per gather call, a wide DVE tensor_tensor multiply (bf16
    2x mode) feeds `a` Scalar-engine activation-Copy free-dim accumulates;
    the remaining slots run fused on DVE via scalar_tensor_tensor (1x).
    All accumulating ops are hardware-capped at 1x, so the work is split
    ~60/40 between ACT and DVE to keep both under the DMA roofline.
  - Masked log-softmax + KL epilogue on [128, 128] f32 tiles; per-sample
    losses DMA out; host sums across samples and cores.
"""

import sys

sys.path.insert(0, "/opt/trn_rl_repo")

import numpy as np

NCORES = 8
B = 1024
D = 1024
M = 128
BL = B // NCORES  # 128 samples per core
TEMP = 0.02
NEG = -1e30
OOB = np.int32(2**30)

_CACHE = {}


def _schedule(blk=8, a_low=4, n_low=2):
    """Gather schedule: list of (m0, blk, act_slots) per indirect call.

    Uniform blk keeps the DMA stream paced by the engines, not by Q7
    emission. ACT takes 5 of 8 slots per block (4 on the last `n_low`
    blocks, shrinking the end-of-stream compute lag and balancing
    ACT ~= DVE ~= 100us, both under the ~125us DMA roofline).
    """
    nblk = M // blk
    sched = []
    for i in range(nblk):
        a = a_low if i >= nblk - n_low else 5
        sched.append((i * blk, blk, a))
    return tuple(sched)


def _build_nc(sched=(), rrows=1, bufs=3, use_bf16=True):
    from concourse import bacc, bass, mybir, tile

    f32 = mybir.dt.float32
    bf16 = mybir.dt.bfloat16
    u8 = mybir.dt.uint8
    i32 = mybir.dt.int32
    ALU = mybir.AluOpType
    AF = mybir.ActivationFunctionType
    AX = mybir.AxisListType
    ddt = bf16 if use_bf16 else f32

    ncalls = len(sched)
    max_a = max(a for _, _, a in sched)

    nc = bacc.Bacc("TRN2", target_bir_lowering=False, debug=False, num_devices=NCORES)

    rdocs = nc.dram_tensor("rdocs", [rrows, D], f32, kind="ExternalInput").ap()
    idxs = nc.dram_tensor("idxs", [BL, ncalls], i32, kind="ExternalInput").ap()
    q = nc.dram_tensor("q", [BL, D], f32, kind="ExternalInput").ap()
    traw = nc.dram_tensor("traw", [BL, M], f32, kind="ExternalInput").ap()
    mask = nc.dram_tensor("mask", [BL, M], u8, kind="ExternalInput").ap()
    out = nc.dram_tensor("out", [BL, 1], f32, kind="ExternalOutput").ap()

    from contextlib import ExitStack

    with tile.TileContext(nc) as tc, ExitStack() as ctx:
        consts = ctx.enter_context(tc.tile_pool(name="consts", bufs=1))
        dbig = ctx.enter_context(tc.tile_pool(name="dbig", bufs=bufs))
        scratch = ctx.enter_context(tc.tile_pool(name="scratch", bufs=3))
        sc2_act = ctx.enter_context(tc.tile_pool(name="actout", bufs=3))
        small = ctx.enter_context(tc.tile_pool(name="small", bufs=1))

        # first gather's index column goes in a tiny DMA so descriptors can
        # be emitted as early as possible
        idx0_sb = consts.tile([BL, 1], i32)
        nc.sync.dma_start(out=idx0_sb, in_=idxs[:, 0:1])
        idx_sb = consts.tile([BL, ncalls], i32)
        nc.sync.dma_start(out=idx_sb, in_=idxs)
        traw_sb = consts.tile([BL, M], f32)
        nc.scalar.dma_start(out=traw_sb, in_=traw)
        mask_sb = consts.tile([BL, M], u8)
        nc.scalar.dma_start(out=mask_sb, in_=mask)
        negt = consts.tile([BL, M], f32)
        nc.vector.memset(negt, NEG)

        # q loads f32 via HWDGE; cast to bf16 on DVE so the Q7/SWDGE path
        # has nothing to do before the first gather's descriptors.
        qf_sb = consts.tile([BL, D], f32)
        nc.scalar.dma_start(out=qf_sb, in_=q)
        q_sb = consts.tile([BL, D], ddt)
        nc.vector.tensor_copy(q_sb, qf_sb)

        # q replicated for the widest per-call multiply
        qrep = consts.tile([BL, max_a * D], ddt)
        for r in range(max_a):
            nc.vector.tensor_copy(qrep[:, r * D : (r + 1) * D], q_sb)

        sim_bm = consts.tile([BL, M], f32)  # raw (unscaled) dots

        for ci, (m0, blk, a) in enumerate(sched):
            # NOTE: out AP must be 2D - a 3D out tile miscompiles the
            # indirect descriptors (probe3). Full 128-partition gathers keep
            # descriptors balanced; OOB-skipped descriptors are ~free.
            dtile = dbig.tile([BL, blk * D], ddt, tag="dbig")
            nc.gpsimd.indirect_dma_start(
                out=dtile,
                out_offset=None,
                in_=rdocs,
                in_offset=bass.IndirectOffsetOnAxis(
                    ap=(idx0_sb[:, 0:1] if ci == 0 else idx_sb[:, ci : ci + 1]),
                    axis=0,
                ),
                bounds_check=rrows - 1,
                oob_is_err=False,
            )
            # one wide bf16 multiply (DVE 2x) feeds `a` ACT accumulates;
            # remaining slots run fused on DVE (1x).
            sc = scratch.tile([BL, max_a * D], ddt, tag="sc")
            if a > 0:
                nc.vector.tensor_mul(
                    sc[:, : a * D], dtile[:, : a * D], qrep[:, : a * D]
                )
            for j in range(a):
                m = m0 + j
                aout = sc2_act.tile([BL, D], ddt, tag="aout")
                # AF.Identity is a lossy spline (probe4); AF.Copy is exact.
                nc.scalar.activation(
                    aout, sc[:, j * D : (j + 1) * D], AF.Copy,
                    accum_out=sim_bm[:, m : m + 1],
                )
            for j in range(a, blk):
                m = m0 + j
                scf = sc2_act.tile([BL, D], ddt, tag="scf")
                nc.vector.scalar_tensor_tensor(
                    out=scf,
                    in0=dtile[:, j * D : (j + 1) * D],
                    scalar=1.0,
                    in1=q_sb,
                    op0=ALU.mult,
                    op1=ALU.mult,
                    accum_out=sim_bm[:, m : m + 1],
                )

        # ---- epilogue on [b=128, m=128] f32 tiles ----
        simt = small.tile([BL, M], f32)
        nc.vector.tensor_scalar(simt, sim_bm, 1.0 / TEMP, None, op0=ALU.mult)
        simm = small.tile([BL, M], f32)
        nc.vector.select(simm, mask_sb, simt, negt)

        nmx = small.tile([BL, 1], f32)
        nc.vector.tensor_reduce(nmx, simm, axis=AX.X, op=ALU.max, negate=True)
        shifted = small.tile([BL, M], f32)
        nc.vector.tensor_scalar_add(shifted, simm, nmx[:, 0:1])

        e_sb = small.tile([BL, M], f32)
        s_sb = small.tile([BL, 1], f32)
        nc.scalar.activation(e_sb, shifted, AF.Exp, accum_out=s_sb)
        logs = small.tile([BL, 1], f32)
        nc.scalar.activation(logs, s_sb, AF.Ln)

        tsum = small.tile([BL, 1], f32)
        nc.vector.tensor_reduce(tsum, traw_sb, axis=AX.X, op=ALU.add)
        denom = small.tile([BL, 1], f32)
        nc.vector.tensor_scalar_add(denom, tsum, 1e-9)
        rec = small.tile([BL, 1], f32)
        nc.vector.reciprocal(rec, denom)
        tn = small.tile([BL, M], f32)
        nc.vector.tensor_scalar_mul(tn, traw_sb, rec[:, 0:1])
        sumtn = small.tile([BL, 1], f32)
        nc.vector.tensor_mul(sumtn, tsum, rec)

        iszero = small.tile([BL, M], f32)
        nc.vector.tensor_scalar(iszero, tn, 0.0, None, op0=ALU.is_le)
        tsafe = small.tile([BL, M], f32)
        nc.vector.tensor_add(tsafe, tn, iszero)
        logt = small.tile([BL, M], f32)
        nc.scalar.activation(logt, tsafe, AF.Ln)

        sc2 = small.tile([BL, M], f32)
        term1 = small.tile([BL, 1], f32)
        nc.vector.scalar_tensor_tensor(
            out=sc2, in0=tn, scalar=1.0, in1=logt,
            op0=ALU.mult, op1=ALU.mult, accum_out=term1,
        )
        sc3 = small.tile([BL, M], f32)
        term2 = small.tile([BL, 1], f32)
        nc.vector.scalar_tensor_tensor(
            out=sc3, in0=tn, scalar=1.0, in1=shifted,
            op0=ALU.mult, op1=ALU.mult, accum_out=term2,
        )

        lgs = small.tile([BL, 1], f32)
        nc.vector.tensor_mul(lgs, logs, sumtn)
        kc = small.tile([BL, 1], f32)
        nc.vector.tensor_sub(kc, term1, term2)
        nc.vector.tensor_add(kc, kc, lgs)
        nc.sync.dma_start(out=out, in_=kc)

    nc.compile()
    return nc


def _get_nc(**cfg):
    key = ("nc",) + tuple(sorted(cfg.items()))
    if key not in _CACHE:
        _CACHE[key] = _build_nc(**cfg)
    return _CACHE[key]


def _make_in_maps(query_embeds, doc_embeds, soft_labels, num_docs_per_sample,
                  bufs=6, use_bf16=True, blk=8):
    qf = np.ascontiguousarray(np.asarray(query_embeds, dtype=np.float32))
    de = np.ascontiguousarray(np.asarray(doc_embeds, dtype=np.float32))
    sl = np.ascontiguousarray(np.asarray(soft_labels, dtype=np.float32))
    nd = np.asarray(num_docs_per_sample).astype(np.int64)
    total = de.shape[0]

    sched = _schedule(blk)
    max_blk = max(b for _, b, _ in sched)

    offs = np.zeros(B, np.int64)
    offs[1:] = np.cumsum(nd)[:-1]
    # effective (clipped) doc counts, mirroring the reference's clip behaviour
    nde = np.minimum(np.minimum(nd, M), np.maximum(total - offs, 0))
    mask = (np.arange(M)[None, :] < nde[:, None]).astype(np.float32)
    traw = sl * mask

    # per-core contiguous doc-row slices
    base = np.empty(NCORES, np.int64)
    rows = np.empty(NCORES, np.int64)
    for c in range(NCORES):
        s0, s1 = c * BL, (c + 1) * BL - 1
        base[c] = offs[s0]
        rows[c] = offs[s1] + nde[s1] - base[c]
    rrows = int(rows.max()) + max_blk  # padding rows for block overreads

    # per-call block-start indices; OOB sentinel when the call's slot range
    # is entirely past the sample's doc count
    m0s = np.array([m0 for m0, _, _ in sched], np.int64)[None, :]  # [1, ncalls]
    relp = (offs - np.repeat(base, BL))[:, None] + m0s  # [B, ncalls]
    valid = m0s < nde[:, None]
    idx_all = np.where(valid, relp, OOB).astype(np.int32)

    # per-sample fetched rows (for engine load balancing); a valid call
    # always fetches the full blk rows (round-up overreads into padding)
    blks = np.array([blk for _, blk, _ in sched], np.int64)[None, :]
    w_all = (valid * blks).sum(axis=1)

    # Each gather descriptor for partition p lands on SDMA engine
    # 2*((p%32)//4) + p//64. Permute samples per core so per-engine bytes are
    # even, giving the known-slow engine 15 a lighter share.
    rate = np.ones(16)
    rate[15] = 0.72
    gran_eng = np.array([2 * (g % 8) + g // 16 for g in range(32)])
    eng_parts = {e: [] for e in range(16)}
    for g in range(32):
        eng_parts[gran_eng[g]].extend(range(4 * g, 4 * g + 4))

    perm = np.empty(B, np.int64)
    for c in range(NCORES):
        s0 = c * BL
        order = np.argsort(-w_all[s0 : s0 + BL], kind="stable")
        load = np.zeros(16)
        cap = np.full(16, 8)
        assign = {e: [] for e in range(16)}
        for i in order:
            scaled = (load + w_all[s0 + i]) / rate
            scaled[cap == 0] = np.inf
            e = int(np.argmin(scaled))
            assign[e].append(i)
            load[e] += w_all[s0 + i]
            cap[e] -= 1
        for e in range(16):
            for slot, i in enumerate(assign[e]):
                perm[s0 + eng_parts[e][slot]] = s0 + i

    in_maps = []
    for c in range(NCORES):
        s = slice(c * BL, (c + 1) * BL)
        p = perm[s]
        docs_c = np.zeros((rrows, D), np.float32)
        docs_c[: rows[c]] = de[base[c] : base[c] + rows[c]]
        in_maps.append(
            {
                "rdocs": docs_c,
                "idxs": np.ascontiguousarray(idx_all[p]),
                "q": np.ascontiguousarray(qf[p]),
                "traw": np.ascontiguousarray(traw[p]),
                "mask": np.ascontiguousarray(mask[p].astype(np.uint8)),
            }
        )
    cfg = {"sched": sched, "rrows": rrows, "bufs": bufs, "use_bf16": use_bf16}
    return in_maps, cfg


def run(in_maps, cfg=None, trace=False):
    from concourse import bass_utils

    nc = _get_nc(**(cfg or {}))
    return bass_utils.run_bass_kernel_spmd(
        nc, in_maps, list(range(NCORES)), trace=trace
    )


def kernel(query_embeds, doc_embeds, soft_labels, num_docs_per_sample):
    in_maps, cfg = _make_in_maps(
        query_embeds, doc_embeds, soft_labels, num_docs_per_sample
    )
    res = run(in_maps, cfg=cfg)
    tot = sum(float(r["out"].sum()) for r in res.results)
    return np.asarray(tot / B, dtype=np.float32)


# revision 39
# speedup vs baseline: 1.1716x; 1.1046x over previous
"""Trainium2 Bass kernel for nn_DistillLoss (ragged KL distillation loss).

Strategy (data-parallel over batch, 8 NeuronCores):
  - Host: shard B=1024 samples into 8 x 128. Per core, samples are permuted
    so per-SDMA-engine gather bytes are balanced (partition granule of 4 ->
    engine 2*((p%32)//4) + p//64), with the known-slow engine 15 derated.
  - Device (per core): ragged doc segments are fetched straight from the
    core's contiguous doc-row slice with "super-row" indirect DMAs: one
    index per sample gathers `blk` consecutive doc rows per descriptor and
    casts f32 -> bf16 inline (SWDGE). Hybrid schedule: blk=8 for the dense
    first 64 doc slots, blk=4 for the ragged tail (less round-up waste).
  - Dot products: per gather call, a wide DVE tensor_tensor multiply (bf16
    2x mode) feeds `a` Scalar-engine activation-Copy free-dim accumulates;
    the remaining slots run fused on DVE via scalar_tensor_tensor (1x).
    All accumulating ops are hardware-capped at 1x, so the work is split
    ~60/40 between ACT and DVE to keep both under the DMA roofline.
  - Masked log-softmax + KL epilogue on [128, 128] f32 tiles; per-sample
    losses DMA out; host sums across samples and cores.
"""

import sys

sys.path.insert(0, "/opt/trn_rl_repo")

import numpy as np

NCORES = 8
B = 1024
D = 1024
M = 128
BL = B // NCORES  # 128 samples per core
TEMP = 0.02
NEG = -1e30
OOB = np.int32(2**30)

_CACHE = {}


def _schedule(blk=8, a_low=4, n_low=2):
    """Gather schedule: list of (m0, blk, act_slots) per indirect call.

    Uniform blk keeps the DMA stream paced by the engines, not by Q7
    emission. ACT takes 5 of 8 slots per block (4 on the last `n_low`
    blocks, shrinking the end-of-stream compute lag and balancing
    ACT ~= DVE ~= 100us, both under the ~125us DMA roofline).
    """
    nblk = M // blk
    sched = []
    for i in range(nblk):
        a = a_low if i >= nblk - n_low else 5
        sched.append((i * blk, blk, a))
    return tuple(sched)


def _build_nc(sched=(), rrows=1, bufs=3, use_bf16=True):
    from concourse import bacc, bass, bass_isa, mybir, tile

    f32 = mybir.dt.float32
    bf16 = mybir.dt.bfloat16
    u8 = mybir.dt.uint8
    i32 = mybir.dt.int32
    ALU = mybir.AluOpType
    AF = mybir.ActivationFunctionType
    AX = mybir.AxisListType
    ddt = bf16 if use_bf16 else f32

    ncalls = len(sched)
    max_a = max(a for _, _, a in sched)

    nc = bacc.Bacc("TRN2", target_bir_lowering=False, debug=False, num_devices=NCORES)

    rdocs = nc.dram_tensor("rdocs", [rrows, D], f32, kind="ExternalInput").ap()
    idxs = nc.dram_tensor("idxs", [BL, ncalls], i32, kind="ExternalInput").ap()
    q = nc.dram_tensor("q", [BL, D], f32, kind="ExternalInput").ap()
    traw = nc.dram_tensor("traw", [BL, M], f32, kind="ExternalInput").ap()
    mask = nc.dram_tensor("mask", [BL, M], u8, kind="ExternalInput").ap()
    out = nc.dram_tensor("out", [1, 1], f32, kind="ExternalOutput").ap()

    from contextlib import ExitStack

    with tile.TileContext(nc) as tc, ExitStack() as ctx:
        consts = ctx.enter_context(tc.tile_pool(name="consts", bufs=1))
        dbig = ctx.enter_context(tc.tile_pool(name="dbig", bufs=bufs))
        scratch = ctx.enter_context(tc.tile_pool(name="scratch", bufs=3))
        sc2_act = ctx.enter_context(tc.tile_pool(name="actout", bufs=3))
        small = ctx.enter_context(tc.tile_pool(name="small", bufs=1))

        # first gather's index column goes in a tiny DMA so descriptors can
        # be emitted as early as possible
        idx0_sb = consts.tile([BL, 1], i32)
        nc.sync.dma_start(out=idx0_sb, in_=idxs[:, 0:1])
        idx_sb = consts.tile([BL, ncalls], i32)
        nc.sync.dma_start(out=idx_sb, in_=idxs)
        traw_sb = consts.tile([BL, M], f32)
        nc.scalar.dma_start(out=traw_sb, in_=traw)
        mask_sb = consts.tile([BL, M], u8)
        nc.scalar.dma_start(out=mask_sb, in_=mask)
        negt = consts.tile([BL, M], f32)
        nc.vector.memset(negt, NEG)

        # q loads f32 via HWDGE; cast to bf16 on DVE so the Q7/SWDGE path
        # has nothing to do before the first gather's descriptors.
        qf_sb = consts.tile([BL, D], f32)
        nc.scalar.dma_start(out=qf_sb, in_=q)
        q_sb = consts.tile([BL, D], ddt)
        nc.vector.tensor_copy(q_sb, qf_sb)

        # q replicated for the widest per-call multiply
        qrep = consts.tile([BL, max_a * D], ddt)
        for r in range(max_a):
            nc.vector.tensor_copy(qrep[:, r * D : (r + 1) * D], q_sb)

        sim_bm = consts.tile([BL, M], f32)  # raw (unscaled) dots

        for ci, (m0, blk, a) in enumerate(sched):
            # NOTE: out AP must be 2D - a 3D out tile miscompiles the
            # indirect descriptors (probe3). Full 128-partition gathers keep
            # descriptors balanced; OOB-skipped descriptors are ~free.
            dtile = dbig.tile([BL, blk * D], ddt, tag="dbig")
            nc.gpsimd.indirect_dma_start(
                out=dtile,
                out_offset=None,
                in_=rdocs,
                in_offset=bass.IndirectOffsetOnAxis(
                    ap=(idx0_sb[:, 0:1] if ci == 0 else idx_sb[:, ci : ci + 1]),
                    axis=0,
                ),
                bounds_check=rrows - 1,
                oob_is_err=False,
            )
            # one wide bf16 multiply (DVE 2x) feeds `a` ACT accumulates;
            # remaining slots run fused on DVE (1x).
            sc = scratch.tile([BL, max_a * D], ddt, tag="sc")
            if a > 0:
                nc.vector.tensor_mul(
                    sc[:, : a * D], dtile[:, : a * D], qrep[:, : a * D]
                )
            for j in range(a):
                m = m0 + j
                aout = sc2_act.tile([BL, D], ddt, tag="aout")
                # AF.Identity is a lossy spline (probe4); AF.Copy is exact.
                nc.scalar.activation(
                    aout, sc[:, j * D : (j + 1) * D], AF.Copy,
                    accum_out=sim_bm[:, m : m + 1],
                )
            for j in range(a, blk):
                m = m0 + j
                scf = sc2_act.tile([BL, D], ddt, tag="scf")
                nc.vector.scalar_tensor_tensor(
                    out=scf,
                    in0=dtile[:, j * D : (j + 1) * D],
                    scalar=1.0,
                    in1=q_sb,
                    op0=ALU.mult,
                    op1=ALU.mult,
                    accum_out=sim_bm[:, m : m + 1],
                )

        # ---- epilogue on [b=128, m=128] f32 tiles ----
        simt = small.tile([BL, M], f32)
        nc.vector.tensor_scalar(simt, sim_bm, 1.0 / TEMP, None, op0=ALU.mult)
        simm = small.tile([BL, M], f32)
        nc.vector.select(simm, mask_sb, simt, negt)

        nmx = small.tile([BL, 1], f32)
        nc.vector.tensor_reduce(nmx, simm, axis=AX.X, op=ALU.max, negate=True)
        shifted = small.tile([BL, M], f32)
        nc.vector.tensor_scalar_add(shifted, simm, nmx[:, 0:1])

        e_sb = small.tile([BL, M], f32)
        s_sb = small.tile([BL, 1], f32)
        nc.scalar.activation(e_sb, shifted, AF.Exp, accum_out=s_sb)
        logs = small.tile([BL, 1], f32)
        nc.scalar.activation(logs, s_sb, AF.Ln)

        tsum = small.tile([BL, 1], f32)
        nc.vector.tensor_reduce(tsum, traw_sb, axis=AX.X, op=ALU.add)
        denom = small.tile([BL, 1], f32)
        nc.vector.tensor_scalar_add(denom, tsum, 1e-9)
        rec = small.tile([BL, 1], f32)
        nc.vector.reciprocal(rec, denom)
        tn = small.tile([BL, M], f32)
        nc.vector.tensor_scalar_mul(tn, traw_sb, rec[:, 0:1])
        sumtn = small.tile([BL, 1], f32)
        nc.vector.tensor_mul(sumtn, tsum, rec)

        iszero = small.tile([BL, M], f32)
        nc.vector.tensor_scalar(iszero, tn, 0.0, None, op0=ALU.is_le)
        tsafe = small.tile([BL, M], f32)
        nc.vector.tensor_add(tsafe, tn, iszero)
        logt = small.tile([BL, M], f32)
        nc.scalar.activation(logt, tsafe, AF.Ln)

        sc2 = small.tile([BL, M], f32)
        term1 = small.tile([BL, 1], f32)
        nc.vector.scalar_tensor_tensor(
            out=sc2, in0=tn, scalar=1.0, in1=logt,
            op0=ALU.mult, op1=ALU.mult, accum_out=term1,
        )
        sc3 = small.tile([BL, M], f32)
        term2 = small.tile([BL, 1], f32)
        nc.vector.scalar_tensor_tensor(
            out=sc3, in0=tn, scalar=1.0, in1=shifted,
            op0=ALU.mult, op1=ALU.mult, accum_out=term2,
        )

        lgs = small.tile([BL, 1], f32)
        nc.vector.tensor_mul(lgs, logs, sumtn)
        kc = small.tile([BL, 1], f32)
        nc.vector.tensor_sub(kc, term1, term2)
        nc.vector.tensor_add(kc, kc, lgs)

        # single-scalar output: [128,1] -> [1,1] via the Q7 daisy chain;
        # a [128,1] out DMA would emit 128 four-byte descriptors.
        tot = small.tile([128, 1], f32)
        nc.gpsimd.partition_all_reduce(
            tot, kc, channels=128, reduce_op=bass_isa.ReduceOp.add
        )
        nc.sync.dma_start(out=out, in_=tot[0:1, 0:1])

    nc.compile()
    return nc


def _get_nc(**cfg):
    key = ("nc",) + tuple(sorted(cfg.items()))
    if key not in _CACHE:
        _CACHE[key] = _build_nc(**cfg)
    return _CACHE[key]


def _make_in_maps(query_embeds, doc_embeds, soft_labels, num_docs_per_sample,
                  bufs=6, use_bf16=True, blk=8):
    qf = np.ascontiguousarray(np.asarray(query_embeds, dtype=np.float32))
    de = np.ascontiguousarray(np.asarray(doc_embeds, dtype=np.float32))
    sl = np.ascontiguousarray(np.asarray(soft_labels, dtype=np.float32))
    nd = np.asarray(num_docs_per_sample).astype(np.int64)
    total = de.shape[0]

    sched = _schedule(blk)
    max_blk = max(b for _, b, _ in sched)

    offs = np.zeros(B, np.int64)
    offs[1:] = np.cumsum(nd)[:-1]
    # effective (clipped) doc counts, mirroring the reference's clip behaviour
    nde = np.minimum(np.minimum(nd, M), np.maximum(total - offs, 0))
    mask = (np.arange(M)[None, :] < nde[:, None]).astype(np.float32)
    traw = sl * mask

    # per-core contiguous doc-row slices
    base = np.empty(NCORES, np.int64)
    rows = np.empty(NCORES, np.int64)
    for c in range(NCORES):
        s0, s1 = c * BL, (c + 1) * BL - 1
        base[c] = offs[s0]
        rows[c] = offs[s1] + nde[s1] - base[c]
    rrows = int(rows.max()) + max_blk  # padding rows for block overreads

    # per-call block-start indices; OOB sentinel when the call's slot range
    # is entirely past the sample's doc count
    m0s = np.array([m0 for m0, _, _ in sched], np.int64)[None, :]  # [1, ncalls]
    relp = (offs - np.repeat(base, BL))[:, None] + m0s  # [B, ncalls]
    valid = m0s < nde[:, None]
    idx_all = np.where(valid, relp, OOB).astype(np.int32)

    # per-sample fetched rows (for engine load balancing); a valid call
    # always fetches the full blk rows (round-up overreads into padding)
    blks = np.array([blk for _, blk, _ in sched], np.int64)[None, :]
    w_all = (valid * blks).sum(axis=1)

    # Each gather descriptor for partition p lands on SDMA engine
    # 2*((p%32)//4) + p//64. Permute samples per core so per-engine bytes are
    # even, giving the known-slow engine 15 a lighter share.
    rate = np.ones(16)
    rate[15] = 0.72
    gran_eng = np.array([2 * (g % 8) + g // 16 for g in range(32)])
    eng_parts = {e: [] for e in range(16)}
    for g in range(32):
        eng_parts[gran_eng[g]].extend(range(4 * g, 4 * g + 4))

    perm = np.empty(B, np.int64)
    for c in range(NCORES):
        s0 = c * BL
        order = np.argsort(-w_all[s0 : s0 + BL], kind="stable")
        load = np.zeros(16)
        cap = np.full(16, 8)
        assign = {e: [] for e in range(16)}
        for i in order:
            scaled = (load + w_all[s0 + i]) / rate
            scaled[cap == 0] = np.inf
            e = int(np.argmin(scaled))
            assign[e].append(i)
            load[e] += w_all[s0 + i]
            cap[e] -= 1
        for e in range(16):
            for slot, i in enumerate(assign[e]):
                perm[s0 + eng_parts[e][slot]] = s0 + i

    in_maps = []
    for c in range(NCORES):
        s = slice(c * BL, (c + 1) * BL)
        p = perm[s]
        docs_c = np.zeros((rrows, D), np.float32)
        docs_c[: rows[c]] = de[base[c] : base[c] + rows[c]]
        in_maps.append(
            {
                "rdocs": docs_c,
                "idxs": np.ascontiguousarray(idx_all[p]),
                "q": np.ascontiguousarray(qf[p]),
                "traw": np.ascontiguousarray(traw[p]),
                "mask": np.ascontiguousarray(mask[p].astype(np.uint8)),
            }
        )
    cfg = {"sched": sched, "rrows": rrows, "bufs": bufs, "use_bf16": use_bf16}
    return in_maps, cfg


def run(in_maps, cfg=None, trace=False):
    from concourse import bass_utils

    nc = _get_nc(**(cfg or {}))
    return bass_utils.run_bass_kernel_spmd(
        nc, in_maps, list(range(NCORES)), trace=trace
    )


def kernel(query_embeds, doc_embeds, soft_labels, num_docs_per_sample):
    in_maps, cfg = _make_in_maps(
        query_embeds, doc_embeds, soft_labels, num_docs_per_sample
    )
    res = run(in_maps, cfg=cfg)
    tot = sum(float(r["out"][0, 0]) for r in res.results)
    return np.asarray(tot / B, dtype=np.float32)
